# Initial kernel scaffold
#
"""Causal self-attention (B=4, T=2048, D=1024, H=16) on 8 Trainium2 cores.

Sharding: core c handles batch b = c // 2 and head-half = c % 2 (8 of the 16
heads). Zero cross-core communication: each core computes q/k/v projections
for its 8 heads, causal flash-style attention, and a partial output
projection against its half of w_o. The host sums the two partial
projections per batch.

Layouts (chosen so every matmul operand is a direct slice, no on-device
transposes):
  xT    (1024, 2048)  x[b].T            rhs of q/k (d on partitions), lhsT of v
  wqT   (1024, 512)   (0.125*w_q[rows]).T  (scale folded in, exact pow2)
  wkT   (1024, 512)   w_k[rows].T
  wvT   (1024, 512)   w_v[rows].T
  woT   (512, 1024)   w_o[:, cols].T
  poutT (1024, 2048)  partial (x @ w_o.T contribution).T

Attention math per head (dh=64): scores are computed TRANSPOSED
(k on partitions, q on free dim) so that softmax(score) tiles feed the
P@V matmul directly as the moving operand. Softmax uses no max-subtraction
(scores are O(5), fp32 exp is safe); the denominator is produced by an
extra all-ones column appended to v (M=65 in the P@V matmul), and the
normalization happens on the 64-row output via reciprocal + partition
broadcast (DMA) + multiply.
"""
import sys

if "/opt/trn_rl_repo" not in sys.path:
    sys.path.insert(0, "/opt/trn_rl_repo")

import numpy as np

B, T, D, H = 4, 2048, 1024, 16
P, TQ = 128, 512
ND = D // P          # 8  d-slices (contraction tiles for projections)
NHP = 4              # head-pairs per core (8 heads)
NQB = T // TQ        # 4  q blocks
NKB = T // P         # 16 k tiles

_COMPILED = None


def _build():
    import concourse.bacc as bacc
    import concourse.tile as tile
    from concourse import mybir
    from contextlib import ExitStack

    F32 = mybir.dt.float32
    F32R = mybir.dt.float32r
    AF = mybir.ActivationFunctionType

    nc = bacc.Bacc("TRN2", target_bir_lowering=False, debug=False, num_devices=8)

    xT = nc.dram_tensor("xT", [D, T], F32, kind="ExternalInput")
    wqT = nc.dram_tensor("wqT", [D, 512], F32, kind="ExternalInput")
    wkT = nc.dram_tensor("wkT", [D, 512], F32, kind="ExternalInput")
    wvT = nc.dram_tensor("wvT", [D, 512], F32, kind="ExternalInput")
    woT = nc.dram_tensor("woT", [512, D], F32, kind="ExternalInput")
    pout = nc.dram_tensor("poutT", [D, T], F32, kind="ExternalOutput")

    with tile.TileContext(nc) as tc:
        with ExitStack() as ctx:
            q_pool = ctx.enter_context(tc.tile_pool(name="q", bufs=NHP))
            k_pool = ctx.enter_context(tc.tile_pool(name="k", bufs=NHP))
            v_pool = ctx.enter_context(tc.tile_pool(name="v", bufs=NKB))
            qT = [q_pool.tile([P, T], F32) for _ in range(NHP)]
            kT = [k_pool.tile([P, T], F32) for _ in range(NHP)]
            # v, row-major (k-position on partitions), 65th column = 1.0
            vA = [v_pool.tile([P, 8, 65], F32) for _ in range(NKB)]

            # ---------------- q/k/v projections ----------------
            with tc.tile_pool(name="xt", bufs=ND) as xt_pool, \
                 tc.tile_pool(name="w", bufs=12) as w_pool, \
                 tc.tile_pool(name="mmps", bufs=4, space="PSUM") as mm_psum:
                xt = []
                for ds in range(ND):
                    t = xt_pool.tile([P, T], F32, tag="xt")
                    nc.sync.dma_start(t, xT[ds * P:(ds + 1) * P, :])
                    xt.append(t)

                # q and k: out tiles are (dh-channel, t) = head-transposed
                for w_dram, outs in ((wqT, qT), (wkT, kT)):
                    wts = []
                    for ds in range(ND):
                        wt = w_pool.tile([P, 512], F32, tag="w")
                        nc.sync.dma_start(wt, w_dram[ds * P:(ds + 1) * P, :])
                        wts.append(wt)
                    for hp in range(NHP):
                        for tt in range(NQB):
                            ps = mm_psum.tile([P, TQ], F32, tag="mm")
                            for ds in range(ND):
                                nc.tensor.matmul(
                                    ps,
                                    wts[ds][:, hp * P:(hp + 1) * P].bitcast(F32R),
                                    xt[ds][:, tt * TQ:(tt + 1) * TQ].bitcast(F32R),
                                    start=(ds == 0), stop=(ds == ND - 1))
                            nc.vector.tensor_copy(
                                outs[hp][:, tt * TQ:(tt + 1) * TQ], ps[:])

                # v: row-major (t on partitions, channel on free)
                wts = []
                for ds in range(ND):
                    wt = w_pool.tile([P, 512], F32, tag="w")
                    nc.sync.dma_start(wt, wvT[ds * P:(ds + 1) * P, :])
                    wts.append(wt)
                for kb in range(NKB):
                    ps = mm_psum.tile([P, TQ], F32, tag="mm")
                    for ds in range(ND):
                        nc.tensor.matmul(
                            ps,
                            xt[ds][:, kb * P:(kb + 1) * P].bitcast(F32R),
                            wts[ds][:].bitcast(F32R),
                            start=(ds == 0), stop=(ds == ND - 1))
                    nc.vector.tensor_copy(
                        vA[kb][:, :, 0:64],
                        ps[:].rearrange("p (h c) -> p h c", c=64))
                    nc.gpsimd.memset(vA[kb][:, :, 64:65], 1.0)

            # ---------------- attention ----------------
            ao_pool = ctx.enter_context(tc.tile_pool(name="ao", bufs=NHP))
            aoT = [ao_pool.tile([P, T], F32) for _ in range(NHP)]
            with tc.tile_pool(name="p", bufs=6) as p_pool, \
                 tc.tile_pool(name="r", bufs=8) as r_pool, \
                 tc.tile_pool(name="sps", bufs=2, space="PSUM") as s_psum, \
                 tc.tile_pool(name="ops", bufs=4, space="PSUM") as o_psum:
                for hp in range(NHP):
                    for qb in range(NQB):
                        nkb = 4 * qb + 4   # causal: k tiles with k0 <= q0+511
                        o_ps = [o_psum.tile([P, TQ], F32, tag="o")
                                for _ in range(2)]
                        for kb in range(nkb):
                            # scores transposed: (k position, q position)
                            s_ps = s_psum.tile([P, 2, TQ], F32, tag="s")
                            for j in range(2):
                                nc.tensor.matmul(
                                    s_ps[:, j, :],
                                    kT[hp][j * 64:(j + 1) * 64,
                                           kb * P:(kb + 1) * P].bitcast(F32R),
                                    qT[hp][j * 64:(j + 1) * 64,
                                           qb * TQ:(qb + 1) * TQ].bitcast(F32R),
                                    tile_position=(j * 64, 0))
                            pt = p_pool.tile([P, 2, TQ], F32, tag="p")
                            nc.scalar.activation(pt[:], s_ps[:], AF.Exp)
                            d = qb * TQ - kb * P   # q0 - k0
                            if d <= 0:
                                # diagonal tile: zero the (q < k) entries
                                nc.gpsimd.affine_select(
                                    out=pt[:], in_=pt[:],
                                    pattern=[[0, 2], [1, TQ]],
                                    compare_op=mybir.AluOpType.is_ge,
                                    fill=0.0, base=d, channel_multiplier=-1)
                            for j in range(2):
                                nc.tensor.matmul(
                                    o_ps[j][0:65, :],
                                    vA[kb][:, 2 * hp + j, :].bitcast(F32R),
                                    pt[:, j, :].bitcast(F32R),
                                    start=(kb == 0), stop=(kb == nkb - 1))
                        for j in range(2):
                            # rows 0..63 = unnormalized out.T, row 64 = sum(exp)
                            r = r_pool.tile([1, TQ], F32, tag="r")
                            nc.vector.reciprocal(r[:], o_ps[j][64:65, :])
                            R = r_pool.tile([64, TQ], F32, tag="R")
                            nc.sync.dma_start(R[:], r[:].to_broadcast((64, TQ)))
                            nc.vector.tensor_mul(
                                aoT[hp][j * 64:(j + 1) * 64,
                                        qb * TQ:(qb + 1) * TQ],
                                o_ps[j][0:64, :], R[:])

            # ---------------- output projection (partial) ----------------
            with tc.tile_pool(name="wo", bufs=4) as wo_pool, \
                 tc.tile_pool(name="pps", bufs=4, space="PSUM") as p_psum:
                wos = []
                for cs in range(4):
                    wt = wo_pool.tile([P, D], F32, tag="wo")
                    nc.sync.dma_start(wt, woT[cs * P:(cs + 1) * P, :])
                    wos.append(wt)
                for od in range(ND):
                    for tt in range(NQB):
                        ps = p_psum.tile([P, TQ], F32, tag="pp")
                        for cs in range(4):
                            nc.tensor.matmul(
                                ps,
                                wos[cs][:, od * P:(od + 1) * P].bitcast(F32R),
                                aoT[cs][:, tt * TQ:(tt + 1) * TQ].bitcast(F32R),
                                start=(cs == 0), stop=(cs == 3))
                        nc.sync.dma_start(
                            pout[od * P:(od + 1) * P, tt * TQ:(tt + 1) * TQ],
                            ps[:])

    nc.compile()
    return nc


def _get_compiled():
    global _COMPILED
    if _COMPILED is None:
        _COMPILED = _build()
    return _COMPILED


def kernel(x, w_q, w_k, w_v, w_o):
    from concourse.bass_utils import run_bass_kernel_spmd

    x = np.asarray(x, dtype=np.float32)
    w_q = np.asarray(w_q, dtype=np.float32)
    w_k = np.asarray(w_k, dtype=np.float32)
    w_v = np.asarray(w_v, dtype=np.float32)
    w_o = np.asarray(w_o, dtype=np.float32)

    nc = _get_compiled()

    xTs = [np.ascontiguousarray(x[b].T) for b in range(B)]
    in_maps = []
    for c in range(8):
        b, half = divmod(c, 2)
        rows = slice(half * 512, (half + 1) * 512)
        in_maps.append({
            "xT": xTs[b],
            "wqT": np.ascontiguousarray((w_q[rows] * 0.125).T),
            "wkT": np.ascontiguousarray(w_k[rows].T),
            "wvT": np.ascontiguousarray(w_v[rows].T),
            "woT": np.ascontiguousarray(w_o[:, rows].T),
        })

    res = run_bass_kernel_spmd(nc, in_maps, list(range(8)))

    out = np.empty((B, T, D), dtype=np.float32)
    for b in range(B):
        out[b] = (res.results[2 * b]["poutT"] + res.results[2 * b + 1]["poutT"]).T
    return out


# revision 21
# speedup vs baseline: 1.2824x; 1.2824x over previous
"""Causal self-attention (B=4, T=2048, D=1024, H=16) on 8 Trainium2 cores.

Sharding: core c handles batch b = c // 2 and head-half = c % 2 (8 of the 16
heads). Zero cross-core communication: each core computes q/k/v projections
for its 8 heads, causal flash-style attention, and a partial output
projection against its half of w_o. The host sums the two partial
projections per batch.

Layouts (chosen so every matmul operand is a direct slice, no on-device
transposes):
  xT    (1024, 2048)  x[b].T            rhs of q/k (d on partitions), lhsT of v
  wqT   (1024, 512)   (0.125*w_q[rows]).T  (scale folded in, exact pow2)
  wkT   (1024, 512)   w_k[rows].T
  wvT   (1024, 512)   w_v[rows].T
  woT   (512, 1024)   w_o[:, cols].T
  poutT (1024, 2048)  partial (x @ w_o.T contribution).T

Attention math per head (dh=64): scores are computed TRANSPOSED
(k on partitions, q on free dim) so that softmax(score) tiles feed the
P@V matmul directly as the moving operand. Softmax uses no max-subtraction
(scores are O(5), fp32 exp is safe); the denominator is produced by an
extra all-ones column appended to v (M=65 in the P@V matmul); the
normalization multiplies the 64 output rows by 1/sums, with sums broadcast
across partitions via a K=1 ones matmul and inverted by
reciprocal_approx_fast (sums >= 1 always: the k=q diagonal term is
exp(|q|^2/8) >= 1).

PRECISION:
  "f32r" - all matmuls in float32r (TF32-like, ~2.8e-4 absmax error)
  "lp"   - q/k/x/w in fp16, softmax weights/v in bf16 (exp can reach e^30,
           beyond fp16 range), PSUM always fp32
"""
import sys

if "/opt/trn_rl_repo" not in sys.path:
    sys.path.insert(0, "/opt/trn_rl_repo")

import numpy as np

B, T, D, H = 4, 2048, 1024, 16
P, TQ = 128, 512
ND = D // P          # 8  d-slices (contraction tiles for projections)
NHP = 4              # head-pairs per core (8 heads)
NQB = T // TQ        # 4  q blocks
NKB = T // P         # 16 k tiles

PRECISION = "f32r"

_COMPILED = {}


def _build(precision):
    import concourse.bacc as bacc
    import concourse.tile as tile
    from concourse import mybir
    from contextlib import ExitStack

    F32 = mybir.dt.float32
    F32R = mybir.dt.float32r
    AF = mybir.ActivationFunctionType

    lp = precision == "lp"
    DT_IN = mybir.dt.float16 if lp else F32   # dram input dtype
    DT_X = mybir.dt.float16 if lp else F32R   # x / wq / wk / wv / wo / q / k / ao
    DT_P = mybir.dt.bfloat16 if lp else F32R  # softmax weights and v

    def dma_in(dst, src_ap):
        # f32r tiles are fed by bitcasting the f32 dram source; fp16 matches
        nc.sync.dma_start(dst, src_ap.bitcast(DT_X) if not lp else src_ap)

    nc = bacc.Bacc("TRN2", target_bir_lowering=False, debug=False, num_devices=8)

    xT = nc.dram_tensor("xT", [D, T], DT_IN, kind="ExternalInput")
    wqT = nc.dram_tensor("wqT", [D, 512], DT_IN, kind="ExternalInput")
    wkT = nc.dram_tensor("wkT", [D, 512], DT_IN, kind="ExternalInput")
    wvT = nc.dram_tensor("wvT", [D, 512], DT_IN, kind="ExternalInput")
    woT = nc.dram_tensor("woT", [512, D], DT_IN, kind="ExternalInput")
    pout = nc.dram_tensor("poutT", [D, T], F32, kind="ExternalOutput")

    with tile.TileContext(nc) as tc:
        with ExitStack() as ctx:
            q_pool = ctx.enter_context(tc.tile_pool(name="q", bufs=NHP))
            k_pool = ctx.enter_context(tc.tile_pool(name="k", bufs=NHP))
            v_pool = ctx.enter_context(tc.tile_pool(name="v", bufs=NKB))
            qT = [q_pool.tile([P, T], DT_X, tag="qT", name="qT") for _ in range(NHP)]
            kT = [k_pool.tile([P, T], DT_X, tag="kT", name="kT") for _ in range(NHP)]
            # v, row-major (k-position on partitions), 65th column = 1.0
            vA = [v_pool.tile([P, 8, 65], DT_P, tag="vA", name="vA") for _ in range(NKB)]

            # ---------------- q/k/v projections ----------------
            with tc.tile_pool(name="xt", bufs=ND) as xt_pool, \
                 tc.tile_pool(name="w", bufs=17) as w_pool, \
                 tc.tile_pool(name="mmps", bufs=4, space="PSUM") as mm_psum:
                # DMA order: wv + first xT column-chunk first, so the first
                # v matmul groups (which read xT columns 0..127) unblock
                # early; remaining xT chunks and wq/wk follow
                xt = [xt_pool.tile([P, T], DT_X, tag="xt", name="xt")
                      for _ in range(ND)]
                wvs = []
                for ds in range(ND):
                    wt = w_pool.tile([P, 512], DT_X, tag="w", name="w")
                    dma_in(wt, wvT[ds * P:(ds + 1) * P, :])
                    wvs.append(wt)
                for ds in range(ND):
                    dma_in(xt[ds][:, 0:TQ], xT[ds * P:(ds + 1) * P, 0:TQ])
                wqs, wks = [], []
                for w_dram, wts in ((wqT, wqs), (wkT, wks)):
                    for ds in range(ND):
                        wt = w_pool.tile([P, 512], DT_X, tag="w", name="w")
                        dma_in(wt, w_dram[ds * P:(ds + 1) * P, :])
                        wts.append(wt)
                for cc in range(1, NQB):
                    for ds in range(ND):
                        dma_in(xt[ds][:, cc * TQ:(cc + 1) * TQ],
                               xT[ds * P:(ds + 1) * P, cc * TQ:(cc + 1) * TQ])
                ones_col = w_pool.tile([P, 8, 1], F32, tag="ones_col",
                                       name="ones_col")
                nc.vector.memset(ones_col[:], 1.0)
                for kb in range(NKB):
                    ps = mm_psum.tile([P, TQ], F32, tag="mm", name="mm")
                    for ds in range(ND):
                        nc.tensor.matmul(
                            ps,
                            xt[ds][:, kb * P:(kb + 1) * P],
                            wvs[ds][:],
                            start=(ds == 0), stop=(ds == ND - 1))
                    nc.vector.tensor_copy(
                        vA[kb][:, :, 0:64],
                        ps[:].rearrange("p (h c) -> p h c", c=64))
                    nc.vector.tensor_copy(vA[kb][:, :, 64:65], ones_col[:])

                # q and k, interleaved per head-pair (attention[hp] unblocks
                # after q[hp]+k[hp])
                for hp in range(NHP):
                    for wts, outs in ((wqs, qT), (wks, kT)):
                        for tt in range(NQB):
                            ps = mm_psum.tile([P, TQ], F32, tag="mm", name="mm")
                            for ds in range(ND):
                                nc.tensor.matmul(
                                    ps,
                                    wts[ds][:, hp * P:(hp + 1) * P],
                                    xt[ds][:, tt * TQ:(tt + 1) * TQ],
                                    start=(ds == 0), stop=(ds == ND - 1))
                            nc.vector.tensor_copy(
                                outs[hp][:, tt * TQ:(tt + 1) * TQ], ps[:])

            # ---------------- attention ----------------
            ao_pool = ctx.enter_context(tc.tile_pool(name="ao", bufs=NHP))
            aoT = [ao_pool.tile([P, T], DT_X, tag="aoT", name="aoT") for _ in range(NHP)]
            with tc.tile_pool(name="p", bufs=6) as p_pool, \
                 tc.tile_pool(name="r", bufs=8) as r_pool, \
                 tc.tile_pool(name="sps", bufs=2, space="PSUM") as s_psum, \
                 tc.tile_pool(name="ops", bufs=3, space="PSUM") as o_psum:
                ones1 = r_pool.tile([1, 64], F32R, tag="ones1", name="ones1")
                ones1f = r_pool.tile([1, 64], F32, tag="ones1f", name="ones1f")
                nc.vector.memset(ones1f[:], 1.0)
                nc.vector.tensor_copy(ones1[:], ones1f[:])
                for hp in range(NHP):
                    for qb in range(NQB):
                        nkb = 4 * qb + 4   # causal: k tiles with k0 <= q0+511
                        o_ps = [o_psum.tile([P, TQ], F32, tag="o", name="o") for _ in range(2)]
                        for kb in range(nkb):
                            # diagonal tiles only need q >= k0: trim the q
                            # range to [qoff, TQ)
                            d = qb * TQ - kb * P   # q0 - k0
                            qoff = max(0, -d)
                            w = TQ - qoff
                            # scores transposed: (k position, q position)
                            s_ps = s_psum.tile([P, 2, TQ], F32, tag="s",
                                               name="s")
                            for j in range(2):
                                nc.tensor.matmul(
                                    s_ps[:, j, 0:w],
                                    kT[hp][j * 64:(j + 1) * 64,
                                           kb * P:(kb + 1) * P],
                                    qT[hp][j * 64:(j + 1) * 64,
                                           qb * TQ + qoff:(qb + 1) * TQ],
                                    tile_position=(j * 64, 0))
                            pt = p_pool.tile([P, 2, TQ], DT_P, tag="p", name="p")
                            nc.scalar.activation(pt[:, :, 0:w], s_ps[:, :, 0:w],
                                                 AF.Exp)
                            if d <= 0:
                                # diagonal tile: zero the (q < k) entries
                                # (trimmed element (p, j, f) has q-k = f - p)
                                nc.gpsimd.affine_select(
                                    out=pt[:, :, 0:w], in_=pt[:, :, 0:w],
                                    pattern=[[0, 2], [1, w]],
                                    compare_op=mybir.AluOpType.is_ge,
                                    fill=0.0, base=0, channel_multiplier=-1)
                            for j in range(2):
                                nc.tensor.matmul(
                                    o_ps[j][0:65, qoff:TQ],
                                    vA[kb][:, 2 * hp + j, :],
                                    pt[:, j, 0:w],
                                    start=(kb == 0), stop=(kb == nkb - 1),
                                    skip_group_check=True)
                        for j in range(2):
                            # rows 0..63 = unnormalized out.T, row 64 = sum(exp)
                            # sum row -> sbuf (f32r for the broadcast matmul)
                            sc = r_pool.tile([1, TQ], F32R, tag="sc", name="sc")
                            nc.vector.tensor_copy(sc[:], o_ps[j][64:65, :])
                            # broadcast sums across 64 partitions via K=1 matmul
                            # (shares a scores-psum slot briefly)
                            rb = o_psum.tile([64, TQ], F32, tag="rb", name="rb", bufs=1)
                            nc.tensor.matmul(rb[:], ones1[:], sc[:],
                                             start=True, stop=True)
                            # 1/sums on all 64 partitions at once (sums >= 1)
                            R = r_pool.tile([64, TQ], F32, tag="R", name="R")
                            nc.vector.reciprocal_approx_fast(R[:], rb[:])
                            nc.vector.tensor_mul(
                                aoT[hp][j * 64:(j + 1) * 64,
                                        qb * TQ:(qb + 1) * TQ],
                                o_ps[j][0:64, :], R[:])

            # ---------------- output projection (partial) ----------------
            with tc.tile_pool(name="wo", bufs=4) as wo_pool, \
                 tc.tile_pool(name="po", bufs=4) as po_pool, \
                 tc.tile_pool(name="pps", bufs=4, space="PSUM") as p_psum:
                wos = []
                for cs in range(4):
                    wt = wo_pool.tile([P, D], DT_X, tag="wo", name="wo")
                    dma_in(wt, woT[cs * P:(cs + 1) * P, :])
                    wos.append(wt)
                for od in range(ND):
                    for tt in range(NQB):
                        ps = p_psum.tile([P, TQ], F32, tag="pp", name="pp")
                        for cs in range(4):
                            nc.tensor.matmul(
                                ps,
                                wos[cs][:, od * P:(od + 1) * P],
                                aoT[cs][:, tt * TQ:(tt + 1) * TQ],
                                start=(cs == 0), stop=(cs == 3))
                        po = po_pool.tile([P, TQ], F32, tag="po", name="po")
                        nc.vector.tensor_copy(po[:], ps[:])
                        nc.sync.dma_start(
                            pout[od * P:(od + 1) * P, tt * TQ:(tt + 1) * TQ],
                            po[:])

    nc.compile()
    return nc


def _get_compiled(precision=None):
    precision = precision or PRECISION
    if precision not in _COMPILED:
        _COMPILED[precision] = _build(precision)
    return _COMPILED[precision]


def make_in_maps(x, w_q, w_k, w_v, w_o, precision=None):
    precision = precision or PRECISION
    dt = np.float16 if precision == "lp" else np.float32
    xTs = [np.ascontiguousarray(x[b].T).astype(dt) for b in range(B)]
    in_maps = []
    for c in range(8):
        b, half = divmod(c, 2)
        rows = slice(half * 512, (half + 1) * 512)
        in_maps.append({
            "xT": xTs[b],
            "wqT": np.ascontiguousarray((w_q[rows] * 0.125).T).astype(dt),
            "wkT": np.ascontiguousarray(w_k[rows].T).astype(dt),
            "wvT": np.ascontiguousarray(w_v[rows].T).astype(dt),
            "woT": np.ascontiguousarray(w_o[:, rows].T).astype(dt),
        })
    return in_maps


def kernel(x, w_q, w_k, w_v, w_o):
    from concourse.bass_utils import run_bass_kernel_spmd

    x = np.asarray(x, dtype=np.float32)
    w_q = np.asarray(w_q, dtype=np.float32)
    w_k = np.asarray(w_k, dtype=np.float32)
    w_v = np.asarray(w_v, dtype=np.float32)
    w_o = np.asarray(w_o, dtype=np.float32)

    nc = _get_compiled()
    in_maps = make_in_maps(x, w_q, w_k, w_v, w_o)
    res = run_bass_kernel_spmd(nc, in_maps, list(range(8)))

    out = np.empty((B, T, D), dtype=np.float32)
    for b in range(B):
        out[b] = (res.results[2 * b]["poutT"] + res.results[2 * b + 1]["poutT"]).T
    return out
